# revision 1
# baseline (speedup 1.0000x reference)
"""Multi-head attention (B=2, S=2048, D=1024, H=16) on 8 NeuronCores.

Sharding: Megatron tensor parallelism. Core r owns heads 2r, 2r+1
(a 128-wide slice of D). Wq/Wk/Wv column-parallel, Wo row-parallel,
ReduceScatter(add) over tokens at the end; host concatenates the 8
token slices and adds bo.

Layouts on device (per core):
  xqT/xkT/xvT : [1024, 4096]  host-transposed activations (feature-major)
  qT/kT       : [128, 2048]   per batch, dk-major (rows = this core's 2 heads)
  v           : [128, 130]    16 token-tiles per batch; cols = [v_h0 | 1 | v_h1 | 1]
                              (ones column makes the PV matmul emit softmax sums)
  scores^T    : psum [128 sk, 512 sq] -> exp on ACT -> PT sbuf
  PV          : psum [65, 512] accumulated over 16 sk tiles; row 64 = sums
  attnT       : [128, 2048]   per batch, normalized, = lhsT for Wo matmul
"""

import sys

sys.path.insert(0, "/opt/trn_rl_repo")

import numpy as np

B, S, D, H, DK = 2, 2048, 1024, 16, 64
NCORES = 8
TOK = B * S            # 4096
DKC = D // NCORES      # 128 = 2 heads per core
TOKC = TOK // NCORES   # 512 output rows per core
KT = D // 128          # 8 contraction tiles
SKT = S // 128         # 16 key tiles per batch
SQB = S // 512         # 4 query blocks per batch

# matmul operand dtype: float32 (exact, 4 cyc/row) or float32r (1 cyc/row)
MM_DT_NAME = "float32r"

_cache = {}


def _build(collective=True):
    from contextlib import ExitStack

    from concourse import bacc
    import concourse.mybir as mybir
    import concourse.tile as tile

    f32 = mybir.dt.float32
    mm_dt = getattr(mybir.dt, MM_DT_NAME)
    Act = mybir.ActivationFunctionType

    def c(ap):
        # bitcast DRAM sources feeding matmul-operand tiles to the matmul dtype
        return ap.bitcast(mm_dt) if mm_dt != f32 else ap

    nc = bacc.Bacc(
        "TRN2", target_bir_lowering=False, debug=False,
        enable_asserts=False, num_devices=NCORES,
    )

    xqT = nc.dram_tensor("xqT", [D, TOK], f32, kind="ExternalInput").ap()
    xkT = nc.dram_tensor("xkT", [D, TOK], f32, kind="ExternalInput").ap()
    xvT = nc.dram_tensor("xvT", [D, TOK], f32, kind="ExternalInput").ap()
    wq = nc.dram_tensor("wq", [D, DKC], f32, kind="ExternalInput").ap()
    wk = nc.dram_tensor("wk", [D, DKC], f32, kind="ExternalInput").ap()
    wv = nc.dram_tensor("wv", [D, DKC], f32, kind="ExternalInput").ap()
    wo = nc.dram_tensor("wo", [DKC, D], f32, kind="ExternalInput").ap()
    bq = nc.dram_tensor("bq", [DKC, 1], f32, kind="ExternalInput").ap()
    bk = nc.dram_tensor("bk", [DKC, 1], f32, kind="ExternalInput").ap()
    bv = nc.dram_tensor("bv", [1, DKC], f32, kind="ExternalInput").ap()
    out_ext = nc.dram_tensor("out", [TOKC, D], f32, kind="ExternalOutput").ap()

    with tile.TileContext(nc) as tc, ExitStack() as ctx, \
            nc.allow_low_precision("float32r matmul operands, fp32 psum accumulate"):
        wpool = ctx.enter_context(tc.tile_pool(name="w", bufs=1))
        xpool = ctx.enter_context(tc.tile_pool(name="x", bufs=12))
        qkpool = ctx.enter_context(tc.tile_pool(name="qk", bufs=2))
        vpool = ctx.enter_context(tc.tile_pool(name="v", bufs=32))
        ptpool = ctx.enter_context(tc.tile_pool(name="pt", bufs=6))
        atpool = ctx.enter_context(tc.tile_pool(name="at", bufs=2))
        smpool = ctx.enter_context(tc.tile_pool(name="sm", bufs=4))
        opool = ctx.enter_context(tc.tile_pool(name="o", bufs=4))
        ps_mm = ctx.enter_context(tc.tile_pool(name="psmm", bufs=3, space="PSUM"))
        ps_acc = ctx.enter_context(tc.tile_pool(name="psacc", bufs=2, space="PSUM"))
        dram = ctx.enter_context(tc.tile_pool(name="dram", bufs=1, space="DRAM"))

        # ---- constants / weights into SBUF ----
        wq_t, wk_t, wv_t = [], [], []
        for name, src, lst in (("wq", wq, wq_t), ("wk", wk, wk_t), ("wv", wv, wv_t)):
            for k in range(KT):
                t = wpool.tile([128, DKC], mm_dt, tag=f"{name}{k}")
                nc.sync.dma_start(t[:], c(src[k * 128:(k + 1) * 128, :]))
                lst.append(t)
        wo_t = wpool.tile([DKC, D], mm_dt, tag="wo")
        nc.sync.dma_start(wo_t[:], c(wo[:]))
        bq_t = wpool.tile([DKC, 1], f32, tag="bq")
        nc.sync.dma_start(bq_t[:], bq[:])
        bk_t = wpool.tile([DKC, 1], f32, tag="bk")
        nc.sync.dma_start(bk_t[:], bk[:])
        bv_t = wpool.tile([1, DKC], mm_dt, tag="bv")
        nc.sync.dma_start(bv_t[:], c(bv[:]))
        ones_f = wpool.tile([1, 128], f32, tag="onesf")
        nc.gpsimd.memset(ones_f[:], 1.0)
        ones_t = wpool.tile([1, 128], mm_dt, tag="ones")
        nc.vector.tensor_copy(ones_t[:], ones_f[:])
        onescol_f = wpool.tile([128, 1], f32, tag="onescolf")
        nc.gpsimd.memset(onescol_f[:], 1.0)

        partial = dram.tile([TOK, D], f32, tag="partial")
        rs_out = dram.tile([TOKC, D], f32, tag="rsout")

        for b in range(B):
            t0 = b * S
            # ---- q/k projections -> qT_b, kT_b [128, S] (dk-major) ----
            qT_b = qkpool.tile([128, S], mm_dt, tag="qT")
            kT_b = qkpool.tile([128, S], mm_dt, tag="kT")
            for xT, w_list, bias_t, dst in (
                (xqT, wq_t, bq_t, qT_b), (xkT, wk_t, bk_t, kT_b),
            ):
                for blk in range(SQB):
                    ps = ps_mm.tile([128, 512], f32, tag="mm")
                    for k in range(KT):
                        xt = xpool.tile([128, 512], mm_dt, tag="xt")
                        nc.sync.dma_start(
                            xt[:],
                            c(xT[k * 128:(k + 1) * 128,
                                 t0 + blk * 512: t0 + (blk + 1) * 512]),
                        )
                        nc.tensor.matmul(
                            ps[:], lhsT=w_list[k][:], rhs=xt[:],
                            start=(k == 0), stop=(k == KT - 1),
                        )
                    nc.scalar.activation(
                        dst[:, blk * 512:(blk + 1) * 512], ps[:],
                        Act.Identity, bias=bias_t[:, 0:1],
                    )

            # ---- v projection -> 16 tiles [128 tok, 130] ----
            v_tiles = []
            for blk in range(SQB):
                xv_blk = []
                for k in range(KT):
                    xt = xpool.tile([128, 512], mm_dt, tag="xt")
                    nc.sync.dma_start(
                        xt[:],
                        c(xvT[k * 128:(k + 1) * 128,
                              t0 + blk * 512: t0 + (blk + 1) * 512]),
                    )
                    xv_blk.append(xt)
                for mi in range(4):
                    ps = ps_mm.tile([128, DKC], f32, tag="mm")
                    for k in range(KT):
                        nc.tensor.matmul(
                            ps[:], lhsT=xv_blk[k][:, mi * 128:(mi + 1) * 128],
                            rhs=wv_t[k][:], start=(k == 0), stop=False,
                        )
                    nc.tensor.matmul(
                        ps[:], lhsT=ones_t[0:1, :], rhs=bv_t[:],
                        start=False, stop=True,
                    )
                    vt = vpool.tile([128, 130], mm_dt, tag="v")
                    nc.vector.tensor_copy(vt[:, 0:64], ps[:, 0:64])
                    nc.vector.tensor_copy(vt[:, 65:129], ps[:, 64:128])
                    nc.vector.tensor_copy(vt[:, 64:65], onescol_f[:])
                    nc.vector.tensor_copy(vt[:, 129:130], onescol_f[:])
                    v_tiles.append(vt)

            # ---- attention (2 heads) -> attnT_b [128, S] ----
            attnT_b = atpool.tile([128, S], mm_dt, tag="attnT")
            for h in range(2):
                hp = h * 64
                for sq in range(SQB):
                    qs = slice(sq * 512, (sq + 1) * 512)
                    xps = ps_acc.tile([65, 512], f32, tag="acc")
                    for sk in range(SKT):
                        sps = ps_mm.tile([128, 512], f32, tag="mm")
                        nc.tensor.matmul(
                            sps[:],
                            lhsT=kT_b[hp:hp + 64, sk * 128:(sk + 1) * 128],
                            rhs=qT_b[hp:hp + 64, qs],
                            start=True, stop=True,
                        )
                        pt = ptpool.tile([128, 512], mm_dt, tag="pt")
                        nc.scalar.activation(pt[:], sps[:], Act.Exp, scale=0.125)
                        nc.tensor.matmul(
                            xps[:], lhsT=v_tiles[sk][:, h * 65:h * 65 + 65],
                            rhs=pt[:], start=(sk == 0), stop=(sk == SKT - 1),
                        )
                    rec = smpool.tile([1, 512], mm_dt, tag="rec")
                    nc.vector.reciprocal(rec[:], xps[64:65, :])
                    rbp = ps_mm.tile([64, 512], f32, tag="mm")
                    nc.tensor.matmul(
                        rbp[:], lhsT=ones_t[0:1, 0:64], rhs=rec[:],
                        start=True, stop=True,
                    )
                    rb = smpool.tile([64, 512], f32, tag="rb")
                    nc.scalar.copy(rb[:], rbp[:])
                    nc.vector.tensor_mul(
                        attnT_b[hp:hp + 64, qs], xps[0:64, :], rb[:],
                    )

            # ---- output projection partial [S, D] ----
            for m in range(S // 128):
                for n2 in range(2):
                    ops = ps_mm.tile([128, 512], f32, tag="mm")
                    nc.tensor.matmul(
                        ops[:], lhsT=attnT_b[:, m * 128:(m + 1) * 128],
                        rhs=wo_t[:, n2 * 512:(n2 + 1) * 512],
                        start=True, stop=True,
                    )
                    ot = opool.tile([128, 512], f32, tag="ot")
                    nc.vector.tensor_copy(ot[:], ops[:])
                    nc.sync.dma_start(
                        partial[t0 + m * 128: t0 + (m + 1) * 128,
                                n2 * 512:(n2 + 1) * 512],
                        ot[:],
                    )

        if collective:
            nc.gpsimd.collective_compute(
                "ReduceScatter",
                mybir.AluOpType.add,
                replica_groups=[list(range(NCORES))],
                ins=[partial.opt()],
                outs=[rs_out.opt()],
            )
            nc.sync.dma_start(out_ext[:], rs_out[:])
        else:
            nc.sync.dma_start(out_ext[:], partial[0:TOKC, :])

    nc.compile()
    return nc


def _get_nc():
    if "nc" not in _cache:
        _cache["nc"] = _build()
    return _cache["nc"]


def kernel(query, key, value, Wq, bq, Wk, bk, Wv, bv, Wo, bo, trace=False):
    from concourse.bass_utils import run_bass_kernel_spmd

    nc = _get_nc()

    q = np.ascontiguousarray(np.asarray(query, np.float32).reshape(TOK, D).T)
    k = np.ascontiguousarray(np.asarray(key, np.float32).reshape(TOK, D).T)
    v = np.ascontiguousarray(np.asarray(value, np.float32).reshape(TOK, D).T)
    Wq = np.asarray(Wq, np.float32)
    Wk = np.asarray(Wk, np.float32)
    Wv = np.asarray(Wv, np.float32)
    Wo = np.asarray(Wo, np.float32)

    in_maps = []
    for r in range(NCORES):
        sl = slice(r * DKC, (r + 1) * DKC)
        in_maps.append({
            "xqT": q, "xkT": k, "xvT": v,
            "wq": np.ascontiguousarray(Wq[:, sl]),
            "wk": np.ascontiguousarray(Wk[:, sl]),
            "wv": np.ascontiguousarray(Wv[:, sl]),
            "wo": np.ascontiguousarray(Wo[sl, :]),
            "bq": np.ascontiguousarray(np.asarray(bq, np.float32)[sl, None]),
            "bk": np.ascontiguousarray(np.asarray(bk, np.float32)[sl, None]),
            "bv": np.ascontiguousarray(np.asarray(bv, np.float32)[None, sl]),
        })

    res = run_bass_kernel_spmd(nc, in_maps, list(range(NCORES)), trace=trace)
    _cache["last_results"] = res

    out = np.concatenate([res.results[r]["out"] for r in range(NCORES)], axis=0)
    out = out + np.asarray(bo, np.float32)[None, :]
    return out.reshape(B, S, D)



# revision 7
# speedup vs baseline: 1.9360x; 1.9360x over previous
"""Multi-head attention (B=2, S=2048, D=1024, H=16) on 8 NeuronCores.

Sharding: Megatron tensor parallelism. Core r owns heads 2r, 2r+1
(a 128-wide slice of D). Wq/Wk/Wv column-parallel. For the output
projection, attnT (each core's 128 D-rows, all 4096 tokens, bf16) is
exchanged with an AllToAll so every core ends up with all 1024 D-rows
for its 512-token slice; each core then multiplies by the full Wo and
writes its token slice. Host concatenates the 8 slices and adds bo.

All matmul operands are bf16 (1 cyc/row on the PE vs 4 for fp32);
PSUM accumulation stays fp32. Activations are converted to bf16 on
the host, halving HBM reads.

Layouts on device (per core):
  xqT/xkT/xvT : [1024, 4096]  host-transposed bf16 activations
  qT/kT       : [128, 2048]   per batch, dk-major (rows = 2 heads)
  v           : [128, 130]    16 tok-tiles/batch; cols = [v_h0 | 1 | v_h1 | 1]
                              (ones column makes PV emit softmax sums)
  scores      : psum [128 sk, 1024] = 2 sk-tiles -> one Exp -> pt bf16
  PV          : psum [65, 512] accumulated over 16 sk tiles; row 64 = sums
  attnT_all   : [128, 4096]   normalized, token-ordered; AllToAll'd
  a2a bufs    : dram [1024, 512] bf16 (8 chunks of [128, 512])
"""

import sys

sys.path.insert(0, "/opt/trn_rl_repo")

import numpy as np

B, S, D, H, DK = 2, 2048, 1024, 16, 64
NCORES = 8
TOK = B * S            # 4096
DKC = D // NCORES      # 128 = 2 heads per core
TOKC = TOK // NCORES   # 512 output rows per core
KT = D // 128          # 8 contraction tiles
SKT = S // 128         # 16 key tiles per batch
SQB = S // 512         # 4 query blocks per batch
SKG = 2                # sk tiles per exp group

_cache = {}


def _build():
    from contextlib import ExitStack

    from concourse import bacc
    import concourse.mybir as mybir
    import concourse.tile as tile

    f32 = mybir.dt.float32
    bf16 = mybir.dt.bfloat16
    Act = mybir.ActivationFunctionType

    nc = bacc.Bacc(
        "TRN2", target_bir_lowering=False, debug=False,
        enable_asserts=False, num_devices=NCORES,
    )

    xqT = nc.dram_tensor("xqT", [D, TOK], bf16, kind="ExternalInput").ap()
    xkT = nc.dram_tensor("xkT", [D, TOK], bf16, kind="ExternalInput").ap()
    xvT = nc.dram_tensor("xvT", [D, TOK], bf16, kind="ExternalInput").ap()
    wq = nc.dram_tensor("wq", [D, DKC], bf16, kind="ExternalInput").ap()
    wk = nc.dram_tensor("wk", [D, DKC], bf16, kind="ExternalInput").ap()
    wv = nc.dram_tensor("wv", [D, DKC], bf16, kind="ExternalInput").ap()
    wo = nc.dram_tensor("wo", [D, D], bf16, kind="ExternalInput").ap()
    bq = nc.dram_tensor("bq", [DKC, 1], f32, kind="ExternalInput").ap()
    bk = nc.dram_tensor("bk", [DKC, 1], f32, kind="ExternalInput").ap()
    bv = nc.dram_tensor("bv", [1, DKC], bf16, kind="ExternalInput").ap()
    out_ext = nc.dram_tensor("out", [TOKC, D], f32, kind="ExternalOutput").ap()

    with tile.TileContext(nc) as tc, ExitStack() as ctx, \
            nc.allow_low_precision("bf16 matmul operands, fp32 psum accumulate"):
        wpool = ctx.enter_context(tc.tile_pool(name="w", bufs=1))
        xpool = ctx.enter_context(tc.tile_pool(name="x", bufs=2))
        qkpool = ctx.enter_context(tc.tile_pool(name="qk", bufs=2))
        vpool = ctx.enter_context(tc.tile_pool(name="v", bufs=2))
        ptpool = ctx.enter_context(tc.tile_pool(name="pt", bufs=3))
        atpool = ctx.enter_context(tc.tile_pool(name="at", bufs=2))
        smpool = ctx.enter_context(tc.tile_pool(name="sm", bufs=2))
        agpool = ctx.enter_context(tc.tile_pool(name="ag", bufs=1))
        opool = ctx.enter_context(tc.tile_pool(name="o", bufs=2))
        ps_g = ctx.enter_context(tc.tile_pool(name="psg", bufs=2, space="PSUM"))
        ps_mm = ctx.enter_context(tc.tile_pool(name="psmm", bufs=2, space="PSUM"))
        ps_rb = ctx.enter_context(tc.tile_pool(name="psrb", bufs=1, space="PSUM"))
        ps_acc = ctx.enter_context(tc.tile_pool(name="psacc", bufs=1, space="PSUM"))
        dram = ctx.enter_context(tc.tile_pool(name="dram", bufs=1, space="DRAM"))

        # ---- constants / weights into SBUF ----
        wq_t, wk_t, wv_t = [], [], []
        for name, src, lst in (("wq", wq, wq_t), ("wk", wk, wk_t), ("wv", wv, wv_t)):
            for k in range(KT):
                t = wpool.tile([128, DKC], bf16, tag=f"{name}{k}")
                nc.sync.dma_start(t[:], src[k * 128:(k + 1) * 128, :])
                lst.append(t)
        wo_t = []
        for r in range(NCORES):
            t = wpool.tile([128, D], bf16, tag=f"wo{r}")
            nc.sync.dma_start(t[:], wo[r * 128:(r + 1) * 128, :])
            wo_t.append(t)
        bq_t = wpool.tile([DKC, 1], f32, tag="bq")
        nc.sync.dma_start(bq_t[:], bq[:])
        bk_t = wpool.tile([DKC, 1], f32, tag="bk")
        nc.sync.dma_start(bk_t[:], bk[:])
        bv_t = wpool.tile([1, DKC], bf16, tag="bv")
        nc.sync.dma_start(bv_t[:], bv[:])
        ones_t = wpool.tile([1, 128], bf16, tag="ones")
        nc.gpsimd.memset(ones_t[:], 1.0)

        a2a_in = dram.tile([NCORES * DKC, TOKC], bf16, tag="a2ain")
        a2a_out = dram.tile([NCORES * DKC, TOKC], bf16, tag="a2aout")

        for b in range(B):
            t0 = b * S
            # ---- q/k projections -> qT_b, kT_b [128, S] (dk-major) ----
            qT_b = qkpool.tile([128, S], bf16, tag="qT")
            kT_b = qkpool.tile([128, S], bf16, tag="kT")
            for xT, w_list, bias_t, dst in (
                (xqT, wq_t, bq_t, qT_b), (xkT, wk_t, bk_t, kT_b),
            ):
                xts = []
                for k in range(KT):
                    xt = xpool.tile([128, S], bf16, tag=f"x{k}")
                    nc.sync.dma_start(
                        xt[:], xT[k * 128:(k + 1) * 128, t0:t0 + S])
                    xts.append(xt)
                for blk in range(SQB):
                    ps = ps_mm.tile([128, 512], f32, tag="mm")
                    for k in range(KT):
                        nc.tensor.matmul(
                            ps[:], lhsT=w_list[k][:],
                            rhs=xts[k][:, blk * 512:(blk + 1) * 512],
                            start=(k == 0), stop=(k == KT - 1),
                        )
                    nc.vector.tensor_scalar_add(
                        dst[:, blk * 512:(blk + 1) * 512], ps[:], bias_t[:, 0:1])

            # ---- v projection -> 16 tiles [128 tok, 130] ----
            xvs = []
            for k in range(KT):
                xt = xpool.tile([128, S], bf16, tag=f"x{k}")
                nc.sync.dma_start(xt[:], xvT[k * 128:(k + 1) * 128, t0:t0 + S])
                xvs.append(xt)
            v_tiles = []
            for mi in range(SKT):
                ps = ps_mm.tile([128, DKC], f32, tag="mm")
                for k in range(KT):
                    nc.tensor.matmul(
                        ps[:], lhsT=xvs[k][:, mi * 128:(mi + 1) * 128],
                        rhs=wv_t[k][:], start=(k == 0), stop=False,
                    )
                nc.tensor.matmul(
                    ps[:], lhsT=ones_t[0:1, :], rhs=bv_t[:],
                    start=False, stop=True,
                )
                vt = vpool.tile([128, 130], bf16, tag=f"v{mi}")
                nc.vector.tensor_copy(vt[:, 0:64], ps[:, 0:64])
                nc.vector.tensor_copy(vt[:, 65:129], ps[:, 64:128])
                nc.gpsimd.memset(vt[:, 64:65], 1.0)
                nc.gpsimd.memset(vt[:, 129:130], 1.0)
                v_tiles.append(vt)

            # ---- attention (2 heads) -> attnT [128, 2048] per batch ----
            attnT = atpool.tile([128, S], bf16, tag="attnT")
            for sq in range(SQB):
                qs = slice(sq * 512, (sq + 1) * 512)
                for h in range(2):
                    hp = h * 64
                    xps = ps_acc.tile([65, 512], f32, tag="acc")
                    for g in range(SKT // SKG):
                        sg = ps_g.tile([128, 512 * SKG], f32, tag="sg")
                        for i in range(SKG):
                            sk = g * SKG + i
                            nc.tensor.matmul(
                                sg[:, i * 512:(i + 1) * 512],
                                lhsT=kT_b[hp:hp + 64, sk * 128:(sk + 1) * 128],
                                rhs=qT_b[hp:hp + 64, qs],
                                start=True, stop=True,
                            )
                        ptg = ptpool.tile([128, 512 * SKG], bf16, tag="pt")
                        nc.scalar.activation(ptg[:], sg[:], Act.Exp, scale=0.125)
                        for i in range(SKG):
                            sk = g * SKG + i
                            nc.tensor.matmul(
                                xps[:],
                                lhsT=v_tiles[sk][:, h * 65:h * 65 + 65],
                                rhs=ptg[:, i * 512:(i + 1) * 512],
                                start=(sk == 0), stop=(sk == SKT - 1),
                            )
                    rowsum = smpool.tile([1, 512], bf16, tag="rs")
                    nc.scalar.activation(rowsum[:], xps[64:65, :], Act.Identity)
                    rbp = ps_rb.tile([64, 512], f32, tag="rbp")
                    nc.tensor.matmul(
                        rbp[:], lhsT=ones_t[0:1, 0:64], rhs=rowsum[:],
                        start=True, stop=True,
                    )
                    rb = smpool.tile([64, 512], f32, tag="rb")
                    nc.vector.reciprocal(rb[:], rbp[:])
                    nc.vector.tensor_mul(
                        attnT[hp:hp + 64, sq * 512:(sq + 1) * 512],
                        xps[0:64, :], rb[:],
                    )
                j = b * SQB + sq
                nc.sync.dma_start(
                    a2a_in[j * 128:(j + 1) * 128, :],
                    attnT[:, sq * 512:(sq + 1) * 512],
                )

        # ---- exchange attnT slices; each core gets its 512 tokens ----
        nc.gpsimd.collective_compute(
            "AllToAll",
            mybir.AluOpType.bypass,
            replica_groups=[list(range(NCORES))],
            ins=[a2a_in.opt()],
            outs=[a2a_out.opt()],
        )

        # ---- output projection for this core's 512 tokens, full D ----
        agt = []
        for r in range(NCORES):
            t = agpool.tile([128, TOKC], bf16, tag=f"ag{r}")
            nc.sync.dma_start(t[:], a2a_out[r * 128:(r + 1) * 128, :])
            agt.append(t)
        for tch in range(TOKC // 128):
            for half in range(2):
                ps = ps_mm.tile([128, 512], f32, tag="mm")
                for r in range(NCORES):
                    nc.tensor.matmul(
                        ps[:],
                        lhsT=agt[r][:, tch * 128:(tch + 1) * 128],
                        rhs=wo_t[r][:, half * 512:(half + 1) * 512],
                        start=(r == 0), stop=(r == NCORES - 1),
                    )
                ot = opool.tile([128, 512], f32, tag="ot")
                nc.vector.tensor_copy(ot[:], ps[:])
                nc.sync.dma_start(
                    out_ext[tch * 128:(tch + 1) * 128,
                            half * 512:(half + 1) * 512],
                    ot[:],
                )

    nc.compile()
    return nc


def _get_nc():
    if "nc" not in _cache:
        _cache["nc"] = _build()
    return _cache["nc"]


def kernel(query, key, value, Wq, bq, Wk, bk, Wv, bv, Wo, bo, trace=False):
    import ml_dtypes
    from concourse.bass_utils import run_bass_kernel_spmd

    bf = ml_dtypes.bfloat16
    nc = _get_nc()

    q = np.ascontiguousarray(
        np.asarray(query, np.float32).reshape(TOK, D).T.astype(bf))
    k = np.ascontiguousarray(
        np.asarray(key, np.float32).reshape(TOK, D).T.astype(bf))
    v = np.ascontiguousarray(
        np.asarray(value, np.float32).reshape(TOK, D).T.astype(bf))
    Wq = np.asarray(Wq, np.float32)
    Wk = np.asarray(Wk, np.float32)
    Wv = np.asarray(Wv, np.float32)
    Wo_b = np.ascontiguousarray(np.asarray(Wo, np.float32).astype(bf))

    in_maps = []
    for r in range(NCORES):
        sl = slice(r * DKC, (r + 1) * DKC)
        in_maps.append({
            "xqT": q, "xkT": k, "xvT": v,
            "wq": np.ascontiguousarray(Wq[:, sl].astype(bf)),
            "wk": np.ascontiguousarray(Wk[:, sl].astype(bf)),
            "wv": np.ascontiguousarray(Wv[:, sl].astype(bf)),
            "wo": Wo_b,
            "bq": np.ascontiguousarray(np.asarray(bq, np.float32)[sl, None]),
            "bk": np.ascontiguousarray(np.asarray(bk, np.float32)[sl, None]),
            "bv": np.ascontiguousarray(
                np.asarray(bv, np.float32)[None, sl].astype(bf)),
        })

    res = run_bass_kernel_spmd(nc, in_maps, list(range(NCORES)), trace=trace)
    _cache["last_results"] = res

    out = np.concatenate([res.results[r]["out"] for r in range(NCORES)], axis=0)
    out = out + np.asarray(bo, np.float32)[None, :]
    return out.reshape(B, S, D)


# revision 13
# speedup vs baseline: 1.9728x; 1.0190x over previous
"""Multi-head attention (B=2, S=2048, D=1024, H=16) on 8 NeuronCores.

Sharding: Megatron tensor parallelism. Core r owns heads 2r, 2r+1
(a 128-wide slice of D). Wq/Wk/Wv column-parallel. For the output
projection, attnT (each core's 128 D-rows, all 4096 tokens, bf16) is
exchanged with an AllToAll so every core ends up with all 1024 D-rows
for its 512-token slice; each core then multiplies by the full Wo and
writes its token slice. Host concatenates the 8 slices and adds bo.

All matmul operands are bf16 (1 cyc/row on the PE vs 4 for fp32);
PSUM accumulation stays fp32. Activations are converted to bf16 on
the host, halving HBM reads.

Layouts on device (per core):
  xqT/xkT/xvT : [1024, 4096]  host-transposed bf16 activations
  qT/kT       : [128, 2048]   per batch, dk-major (rows = 2 heads)
  v           : [128, 130]    16 tok-tiles/batch; cols = [v_h0 | 1 | v_h1 | 1]
                              (ones column makes PV emit softmax sums)
  scores      : psum [128 sk, 1024] = 2 sk-tiles -> one Exp -> pt bf16
  PV          : psum [65, 512] accumulated over 16 sk tiles; row 64 = sums
  attnT_all   : [128, 4096]   normalized, token-ordered; AllToAll'd
  a2a bufs    : dram [1024, 512] bf16 (8 chunks of [128, 512])
"""

import sys

sys.path.insert(0, "/opt/trn_rl_repo")

import numpy as np

B, S, D, H, DK = 2, 2048, 1024, 16, 64
NCORES = 8
TOK = B * S            # 4096
DKC = D // NCORES      # 128 = 2 heads per core
TOKC = TOK // NCORES   # 512 output rows per core
KT = D // 128          # 8 contraction tiles
SKT = S // 128         # 16 key tiles per batch
SQB = S // 512         # 4 query blocks per batch
SKG = 2                # sk tiles per exp group

_cache = {}


def _build():
    from contextlib import ExitStack

    from concourse import bacc
    import concourse.mybir as mybir
    import concourse.tile as tile

    f32 = mybir.dt.float32
    bf16 = mybir.dt.bfloat16
    Act = mybir.ActivationFunctionType

    nc = bacc.Bacc(
        "TRN2", target_bir_lowering=False, debug=False,
        enable_asserts=False, num_devices=NCORES,
    )

    xqT = nc.dram_tensor("xqT", [D, TOK], bf16, kind="ExternalInput").ap()
    xkT = nc.dram_tensor("xkT", [D, TOK], bf16, kind="ExternalInput").ap()
    xvT = nc.dram_tensor("xvT", [D, TOK], bf16, kind="ExternalInput").ap()
    wq = nc.dram_tensor("wq", [D, DKC], bf16, kind="ExternalInput").ap()
    wk = nc.dram_tensor("wk", [D, DKC], bf16, kind="ExternalInput").ap()
    wv = nc.dram_tensor("wv", [D, DKC], bf16, kind="ExternalInput").ap()
    wo = nc.dram_tensor("wo", [D, D], bf16, kind="ExternalInput").ap()
    bq = nc.dram_tensor("bq", [DKC, 1], f32, kind="ExternalInput").ap()
    bk = nc.dram_tensor("bk", [DKC, 1], f32, kind="ExternalInput").ap()
    bv = nc.dram_tensor("bv", [1, DKC], bf16, kind="ExternalInput").ap()
    out_ext = nc.dram_tensor("out", [TOKC, D], f32, kind="ExternalOutput").ap()

    with tile.TileContext(nc) as tc, ExitStack() as ctx, \
            nc.allow_low_precision("bf16 matmul operands, fp32 psum accumulate"):
        wpool = ctx.enter_context(tc.tile_pool(name="w", bufs=1))
        xpool = ctx.enter_context(tc.tile_pool(name="x", bufs=2))
        qkpool = ctx.enter_context(tc.tile_pool(name="qk", bufs=2))
        vpool = ctx.enter_context(tc.tile_pool(name="v", bufs=2))
        ptpool = ctx.enter_context(tc.tile_pool(name="pt", bufs=3))
        atpool = ctx.enter_context(tc.tile_pool(name="at", bufs=2))
        smpool = ctx.enter_context(tc.tile_pool(name="sm", bufs=2))
        agpool = ctx.enter_context(tc.tile_pool(name="ag", bufs=1))
        opool = ctx.enter_context(tc.tile_pool(name="o", bufs=2))
        ps_g = ctx.enter_context(tc.tile_pool(name="psg", bufs=2, space="PSUM"))
        ps_mm = ctx.enter_context(tc.tile_pool(name="psmm", bufs=2, space="PSUM"))
        ps_acc = ctx.enter_context(tc.tile_pool(name="psacc", bufs=2, space="PSUM"))
        dram = ctx.enter_context(tc.tile_pool(name="dram", bufs=1, space="DRAM"))

        # ---- constants / weights into SBUF ----
        wq_t, wk_t, wv_t = [], [], []
        for name, src, lst in (("wq", wq, wq_t), ("wk", wk, wk_t), ("wv", wv, wv_t)):
            for k in range(KT):
                t = wpool.tile([128, DKC], bf16, tag=f"{name}{k}")
                nc.sync.dma_start(t[:], src[k * 128:(k + 1) * 128, :])
                lst.append(t)
        wo_t = []
        for r in range(NCORES):
            t = wpool.tile([128, D], bf16, tag=f"wo{r}")
            nc.sync.dma_start(t[:], wo[r * 128:(r + 1) * 128, :])
            wo_t.append(t)
        bq_t = wpool.tile([DKC, 1], f32, tag="bq")
        nc.sync.dma_start(bq_t[:], bq[:])
        bk_t = wpool.tile([DKC, 1], f32, tag="bk")
        nc.sync.dma_start(bk_t[:], bk[:])
        bv_t = wpool.tile([1, DKC], bf16, tag="bv")
        nc.sync.dma_start(bv_t[:], bv[:])
        ones_t = wpool.tile([1, 128], bf16, tag="ones")
        nc.gpsimd.memset(ones_t[:], 1.0)

        def emit_chunk(c, attnT, loc0):
            """AllToAll + output projection for 1024 global tokens
            [c*1024, (c+1)*1024); this core ends up owning the 128-token
            span c*1024 + rank*128 and writes out rows c*128:(c+1)*128."""
            ain = dram.tile([NCORES * 128, 128], bf16, tag=f"a2ai{c}")
            aout = dram.tile([NCORES * 128, 128], bf16, tag=f"a2ao{c}")
            for j in range(NCORES):
                nc.sync.dma_start(
                    ain[j * 128:(j + 1) * 128, :],
                    attnT[:, loc0 + j * 128: loc0 + (j + 1) * 128],
                )
            nc.gpsimd.collective_compute(
                "AllToAll",
                mybir.AluOpType.bypass,
                replica_groups=[list(range(NCORES))],
                ins=[ain.opt()],
                outs=[aout.opt()],
            )
            agts = []
            for r in range(NCORES):
                t = agpool.tile([128, 128], bf16, tag=f"ag{c}_{r}")
                nc.sync.dma_start(t[:], aout[r * 128:(r + 1) * 128, :])
                agts.append(t)
            for half in range(2):
                ps = ps_mm.tile([128, 512], f32, tag="mm")
                for r in range(NCORES):
                    nc.tensor.matmul(
                        ps[:], lhsT=agts[r][:],
                        rhs=wo_t[r][:, half * 512:(half + 1) * 512],
                        start=(r == 0), stop=(r == NCORES - 1),
                    )
                ot = opool.tile([128, 512], f32, tag="ot")
                nc.vector.tensor_copy(ot[:], ps[:])
                nc.sync.dma_start(
                    out_ext[c * 128:(c + 1) * 128,
                            half * 512:(half + 1) * 512],
                    ot[:],
                )

        for b in range(B):
            t0 = b * S
            # ---- q/k projections -> qT_b, kT_b [128, S] (dk-major) ----
            qT_b = qkpool.tile([128, S], bf16, tag="qT")
            kT_b = qkpool.tile([128, S], bf16, tag="kT")
            for xT, w_list, bias_t, dst in (
                (xqT, wq_t, bq_t, qT_b), (xkT, wk_t, bk_t, kT_b),
            ):
                xts = []
                for k in range(KT):
                    xt = xpool.tile([128, S], bf16, tag=f"x{k}")
                    nc.sync.dma_start(
                        xt[:], xT[k * 128:(k + 1) * 128, t0:t0 + S])
                    xts.append(xt)
                for blk in range(SQB):
                    ps = ps_mm.tile([128, 512], f32, tag="mm")
                    for k in range(KT):
                        nc.tensor.matmul(
                            ps[:], lhsT=w_list[k][:],
                            rhs=xts[k][:, blk * 512:(blk + 1) * 512],
                            start=(k == 0), stop=(k == KT - 1),
                        )
                    nc.vector.tensor_scalar_add(
                        dst[:, blk * 512:(blk + 1) * 512], ps[:], bias_t[:, 0:1])

            # ---- v projection -> 16 tiles [128 tok, 130] ----
            xvs = []
            for k in range(KT):
                xt = xpool.tile([128, S], bf16, tag=f"x{k}")
                nc.sync.dma_start(xt[:], xvT[k * 128:(k + 1) * 128, t0:t0 + S])
                xvs.append(xt)
            v_tiles = []
            for mi in range(SKT):
                ps = ps_mm.tile([128, DKC], f32, tag="mm")
                for k in range(KT):
                    nc.tensor.matmul(
                        ps[:], lhsT=xvs[k][:, mi * 128:(mi + 1) * 128],
                        rhs=wv_t[k][:], start=(k == 0), stop=False,
                    )
                nc.tensor.matmul(
                    ps[:], lhsT=ones_t[0:1, :], rhs=bv_t[:],
                    start=False, stop=True,
                )
                vt = vpool.tile([128, 130], bf16, tag=f"v{mi}")
                nc.vector.tensor_copy(vt[:, 0:64], ps[:, 0:64])
                nc.vector.tensor_copy(vt[:, 65:129], ps[:, 64:128])
                nc.vector.memset(vt[:, 64:65], 1.0)
                nc.vector.memset(vt[:, 129:130], 1.0)
                v_tiles.append(vt)

            # ---- attention (2 heads) -> attnT [128, 2048] per batch ----
            attnT = atpool.tile([128, S], bf16, tag="attnT")
            for sq in range(SQB):
                qs = slice(sq * 512, (sq + 1) * 512)
                for h in range(2):
                    hp = h * 64
                    xps = ps_acc.tile([65, 512], f32, tag="acc")
                    for g in range(SKT // SKG):
                        sg = ps_g.tile([128, 512 * SKG], f32, tag="sg")
                        for i in range(SKG):
                            sk = g * SKG + i
                            nc.tensor.matmul(
                                sg[:, i * 512:(i + 1) * 512],
                                lhsT=kT_b[hp:hp + 64, sk * 128:(sk + 1) * 128],
                                rhs=qT_b[hp:hp + 64, qs],
                                start=True, stop=True,
                            )
                        ptg = ptpool.tile([128, 512 * SKG], bf16, tag="pt")
                        nc.scalar.activation(ptg[:], sg[:], Act.Exp, scale=0.125)
                        for i in range(SKG):
                            sk = g * SKG + i
                            nc.tensor.matmul(
                                xps[:],
                                lhsT=v_tiles[sk][:, h * 65:h * 65 + 65],
                                rhs=ptg[:, i * 512:(i + 1) * 512],
                                start=(sk == 0), stop=(sk == SKT - 1),
                            )
                    rowsum = smpool.tile([1, 512], bf16, tag="rs")
                    nc.scalar.activation(rowsum[:], xps[64:65, :], Act.Identity)
                    rbp = ps_mm.tile([64, 512], f32, tag="mm")
                    nc.tensor.matmul(
                        rbp[:], lhsT=ones_t[0:1, 0:64], rhs=rowsum[:],
                        start=True, stop=True,
                    )
                    rb = smpool.tile([64, 512], f32, tag="rb")
                    nc.vector.reciprocal_approx_fast(rb[:], rbp[:])
                    nc.vector.tensor_mul(
                        attnT[hp:hp + 64, sq * 512:(sq + 1) * 512],
                        xps[0:64, :], rb[:],
                    )
                if sq % 2 == 1:
                    c = b * 2 + sq // 2
                    emit_chunk(c, attnT, (sq // 2) * 1024)

    nc.compile()
    return nc


def _get_nc():
    if "nc" not in _cache:
        _cache["nc"] = _build()
    return _cache["nc"]


def kernel(query, key, value, Wq, bq, Wk, bk, Wv, bv, Wo, bo, trace=False):
    import ml_dtypes
    from concourse.bass_utils import run_bass_kernel_spmd

    bf = ml_dtypes.bfloat16
    nc = _get_nc()

    q = np.ascontiguousarray(
        np.asarray(query, np.float32).reshape(TOK, D).T.astype(bf))
    k = np.ascontiguousarray(
        np.asarray(key, np.float32).reshape(TOK, D).T.astype(bf))
    v = np.ascontiguousarray(
        np.asarray(value, np.float32).reshape(TOK, D).T.astype(bf))
    Wq = np.asarray(Wq, np.float32)
    Wk = np.asarray(Wk, np.float32)
    Wv = np.asarray(Wv, np.float32)
    Wo_b = np.ascontiguousarray(np.asarray(Wo, np.float32).astype(bf))

    in_maps = []
    for r in range(NCORES):
        sl = slice(r * DKC, (r + 1) * DKC)
        in_maps.append({
            "xqT": q, "xkT": k, "xvT": v,
            "wq": np.ascontiguousarray(Wq[:, sl].astype(bf)),
            "wk": np.ascontiguousarray(Wk[:, sl].astype(bf)),
            "wv": np.ascontiguousarray(Wv[:, sl].astype(bf)),
            "wo": Wo_b,
            "bq": np.ascontiguousarray(np.asarray(bq, np.float32)[sl, None]),
            "bk": np.ascontiguousarray(np.asarray(bk, np.float32)[sl, None]),
            "bv": np.ascontiguousarray(
                np.asarray(bv, np.float32)[None, sl].astype(bf)),
        })

    res = run_bass_kernel_spmd(nc, in_maps, list(range(NCORES)), trace=trace)
    _cache["last_results"] = res

    out = np.empty((TOK, D), np.float32)
    for r in range(NCORES):
        o = np.asarray(res.results[r]["out"])
        for c in range(TOK // 1024):
            out[c * 1024 + r * 128: c * 1024 + (r + 1) * 128] = \
                o[c * 128:(c + 1) * 128]
    out = out + np.asarray(bo, np.float32)[None, :]
    return out.reshape(B, S, D)


# revision 15
# speedup vs baseline: 2.2412x; 1.1360x over previous
"""Multi-head attention (B=2, S=2048, D=1024, H=16) on 8 NeuronCores.

Sharding: Megatron tensor parallelism. Core r owns heads 2r, 2r+1
(a 128-wide slice of D). Wq/Wk/Wv column-parallel. For the output
projection, attnT (each core's 128 D-rows, all 4096 tokens, bf16) is
exchanged with an AllToAll so every core ends up with all 1024 D-rows
for its 512-token slice; each core then multiplies by the full Wo and
writes its token slice. Host concatenates the 8 slices and adds bo.

All matmul operands are bf16 (1 cyc/row on the PE vs 4 for fp32);
PSUM accumulation stays fp32. Activations are converted to bf16 on
the host, halving HBM reads.

Layouts on device (per core):
  xqT/xkT/xvT : [1024, 4096]  host-transposed bf16 activations
  qT/kT       : [128, 2048]   per batch, dk-major (rows = 2 heads)
  v           : [128, 130]    16 tok-tiles/batch; cols = [v_h0 | 1 | v_h1 | 1]
                              (ones column makes PV emit softmax sums)
  scores      : psum [128 sk, 1024] = 2 sk-tiles -> one Exp -> pt bf16
  PV          : psum [65, 512] accumulated over 16 sk tiles; row 64 = sums
  attnT_all   : [128, 4096]   normalized, token-ordered; AllToAll'd
  a2a bufs    : dram [1024, 512] bf16 (8 chunks of [128, 512])
"""

import sys

sys.path.insert(0, "/opt/trn_rl_repo")

import numpy as np

B, S, D, H, DK = 2, 2048, 1024, 16, 64
NCORES = 8
TOK = B * S            # 4096
DKC = D // NCORES      # 128 = 2 heads per core
TOKC = TOK // NCORES   # 512 output rows per core
KT = D // 128          # 8 contraction tiles
SKT = S // 128         # 16 key tiles per batch
SQB = S // 512         # 4 query blocks per batch
SKG = 2                # sk tiles per exp group

_cache = {}


def _build():
    from contextlib import ExitStack

    from concourse import bacc
    import concourse.mybir as mybir
    import concourse.tile as tile

    f32 = mybir.dt.float32
    bf16 = mybir.dt.bfloat16
    Act = mybir.ActivationFunctionType

    nc = bacc.Bacc(
        "TRN2", target_bir_lowering=False, debug=False,
        enable_asserts=False, num_devices=NCORES,
    )

    xqT = nc.dram_tensor("xqT", [D, TOK], bf16, kind="ExternalInput").ap()
    xkT = nc.dram_tensor("xkT", [D, TOK], bf16, kind="ExternalInput").ap()
    xvT = nc.dram_tensor("xvT", [D, TOK], bf16, kind="ExternalInput").ap()
    wq = nc.dram_tensor("wq", [D, DKC], bf16, kind="ExternalInput").ap()
    wk = nc.dram_tensor("wk", [D, DKC], bf16, kind="ExternalInput").ap()
    wv = nc.dram_tensor("wv", [D, DKC], bf16, kind="ExternalInput").ap()
    wo = nc.dram_tensor("wo", [D, D], bf16, kind="ExternalInput").ap()
    bq = nc.dram_tensor("bq", [DKC, 1], f32, kind="ExternalInput").ap()
    bk = nc.dram_tensor("bk", [DKC, 1], f32, kind="ExternalInput").ap()
    bv = nc.dram_tensor("bv", [1, DKC], bf16, kind="ExternalInput").ap()
    out_ext = nc.dram_tensor("out", [TOKC, D], f32, kind="ExternalOutput").ap()

    with tile.TileContext(nc) as tc, ExitStack() as ctx, \
            nc.allow_low_precision("bf16 matmul operands, fp32 psum accumulate"):
        wpool = ctx.enter_context(tc.tile_pool(name="w", bufs=1))
        xpool = ctx.enter_context(tc.tile_pool(name="x", bufs=2))
        qkpool = ctx.enter_context(tc.tile_pool(name="qk", bufs=2))
        vpool = ctx.enter_context(tc.tile_pool(name="v", bufs=2))
        ptpool = ctx.enter_context(tc.tile_pool(name="pt", bufs=3))
        atpool = ctx.enter_context(tc.tile_pool(name="at", bufs=2))
        smpool = ctx.enter_context(tc.tile_pool(name="sm", bufs=2))
        agpool = ctx.enter_context(tc.tile_pool(name="ag", bufs=1))
        opool = ctx.enter_context(tc.tile_pool(name="o", bufs=2))
        ps_g = ctx.enter_context(tc.tile_pool(name="psg", bufs=2, space="PSUM"))
        ps_mm = ctx.enter_context(tc.tile_pool(name="psmm", bufs=2, space="PSUM"))
        ps_acc = ctx.enter_context(tc.tile_pool(name="psacc", bufs=2, space="PSUM"))
        dram = ctx.enter_context(tc.tile_pool(name="dram", bufs=1, space="DRAM"))

        # ---- constants / weights into SBUF (Activation DMA queue, so the
        # SP queue leads with the x tiles the first matmuls need) ----
        bq_t = wpool.tile([DKC, 1], f32, tag="bq")
        nc.scalar.dma_start(bq_t[:], bq[:])
        bk_t = wpool.tile([DKC, 1], f32, tag="bk")
        nc.scalar.dma_start(bk_t[:], bk[:])
        bv_t = wpool.tile([1, DKC], bf16, tag="bv")
        nc.scalar.dma_start(bv_t[:], bv[:])
        wq_t, wk_t, wv_t = [], [], []
        for name, src, lst in (("wq", wq, wq_t), ("wk", wk, wk_t), ("wv", wv, wv_t)):
            for k in range(KT):
                t = wpool.tile([128, DKC], bf16, tag=f"{name}{k}")
                nc.scalar.dma_start(t[:], src[k * 128:(k + 1) * 128, :])
                lst.append(t)
        wo_t = []
        for r in range(NCORES):
            t = wpool.tile([128, D], bf16, tag=f"wo{r}")
            nc.scalar.dma_start(t[:], wo[r * 128:(r + 1) * 128, :])
            wo_t.append(t)
        ones_t = wpool.tile([1, 128], bf16, tag="ones")
        nc.gpsimd.memset(ones_t[:], 1.0)

        def emit_exchange(c, attnT, loc0):
            """AllToAll for 1024 global tokens [c*1024, (c+1)*1024); this
            core ends up owning the 128-token span c*1024 + rank*128."""
            ain = dram.tile([NCORES * 128, 128], bf16, tag=f"a2ai{c}")
            aout = dram.tile([NCORES * 128, 128], bf16, tag=f"a2ao{c}")
            for j in range(NCORES):
                nc.sync.dma_start(
                    ain[j * 128:(j + 1) * 128, :],
                    attnT[:, loc0 + j * 128: loc0 + (j + 1) * 128],
                )
            nc.gpsimd.collective_compute(
                "AllToAll",
                mybir.AluOpType.bypass,
                replica_groups=[list(range(NCORES))],
                ins=[ain.opt()],
                outs=[aout.opt()],
            )
            return aout

        def emit_wo(c, aout):
            """Output projection for the exchanged chunk c; writes out
            rows c*128:(c+1)*128."""
            agts = []
            for r in range(NCORES):
                t = agpool.tile([128, 128], bf16, tag=f"ag{c}_{r}")
                nc.sync.dma_start(t[:], aout[r * 128:(r + 1) * 128, :])
                agts.append(t)
            for half in range(2):
                ps = ps_mm.tile([128, 512], f32, tag="mm")
                for r in range(NCORES):
                    nc.tensor.matmul(
                        ps[:], lhsT=agts[r][:],
                        rhs=wo_t[r][:, half * 512:(half + 1) * 512],
                        start=(r == 0), stop=(r == NCORES - 1),
                    )
                ot = opool.tile([128, 512], f32, tag="ot")
                nc.vector.tensor_copy(ot[:], ps[:])
                nc.sync.dma_start(
                    out_ext[c * 128:(c + 1) * 128,
                            half * 512:(half + 1) * 512],
                    ot[:],
                )

        aouts = {}
        for b in range(B):
            t0 = b * S
            # ---- q/k projections -> qT_b, kT_b [128, S] (dk-major) ----
            qT_b = qkpool.tile([128, S], bf16, tag="qT")
            kT_b = qkpool.tile([128, S], bf16, tag="kT")
            for xT, w_list, bias_t, dst in (
                (xqT, wq_t, bq_t, qT_b), (xkT, wk_t, bk_t, kT_b),
            ):
                xts = []
                for k in range(KT):
                    xt = xpool.tile([128, S], bf16, tag=f"x{k}")
                    nc.sync.dma_start(
                        xt[:], xT[k * 128:(k + 1) * 128, t0:t0 + S])
                    xts.append(xt)
                for blk in range(SQB):
                    ps = ps_mm.tile([128, 512], f32, tag="mm")
                    for k in range(KT):
                        nc.tensor.matmul(
                            ps[:], lhsT=w_list[k][:],
                            rhs=xts[k][:, blk * 512:(blk + 1) * 512],
                            start=(k == 0), stop=(k == KT - 1),
                        )
                    nc.vector.tensor_scalar_add(
                        dst[:, blk * 512:(blk + 1) * 512], ps[:], bias_t[:, 0:1])

            # ---- v projection -> 16 tiles [128 tok, 130] ----
            xvs = []
            for k in range(KT):
                xt = xpool.tile([128, S], bf16, tag=f"x{k}")
                nc.sync.dma_start(xt[:], xvT[k * 128:(k + 1) * 128, t0:t0 + S])
                xvs.append(xt)
            v_tiles = []
            for mi in range(SKT):
                ps = ps_mm.tile([128, DKC], f32, tag="mm")
                for k in range(KT):
                    nc.tensor.matmul(
                        ps[:], lhsT=xvs[k][:, mi * 128:(mi + 1) * 128],
                        rhs=wv_t[k][:], start=(k == 0), stop=False,
                    )
                nc.tensor.matmul(
                    ps[:], lhsT=ones_t[0:1, :], rhs=bv_t[:],
                    start=False, stop=True,
                )
                vt = vpool.tile([128, 130], bf16, tag=f"v{mi}")
                nc.vector.tensor_copy(vt[:, 0:64], ps[:, 0:64])
                nc.vector.tensor_copy(vt[:, 65:129], ps[:, 64:128])
                nc.vector.memset(vt[:, 64:65], 1.0)
                nc.vector.memset(vt[:, 129:130], 1.0)
                v_tiles.append(vt)

            # ---- attention (2 heads) -> attnT [128, 2048] per batch ----
            attnT = atpool.tile([128, S], bf16, tag="attnT")
            for sq in range(SQB):
                qs = slice(sq * 512, (sq + 1) * 512)
                for h in range(2):
                    hp = h * 64
                    xps = ps_acc.tile([65, 512], f32, tag="acc")
                    for g in range(SKT // SKG):
                        sg = ps_g.tile([128, 512 * SKG], f32, tag="sg")
                        for i in range(SKG):
                            sk = g * SKG + i
                            nc.tensor.matmul(
                                sg[:, i * 512:(i + 1) * 512],
                                lhsT=kT_b[hp:hp + 64, sk * 128:(sk + 1) * 128],
                                rhs=qT_b[hp:hp + 64, qs],
                                start=True, stop=True,
                            )
                        ptg = ptpool.tile([128, 512 * SKG], bf16, tag="pt")
                        nc.scalar.activation(ptg[:], sg[:], Act.Exp, scale=0.125)
                        for i in range(SKG):
                            sk = g * SKG + i
                            nc.tensor.matmul(
                                xps[:],
                                lhsT=v_tiles[sk][:, h * 65:h * 65 + 65],
                                rhs=ptg[:, i * 512:(i + 1) * 512],
                                start=(sk == 0), stop=(sk == SKT - 1),
                            )
                    rowsum = smpool.tile([1, 512], bf16, tag="rs")
                    nc.scalar.activation(rowsum[:], xps[64:65, :], Act.Identity)
                    rbp = ps_mm.tile([64, 512], f32, tag="mm")
                    nc.tensor.matmul(
                        rbp[:], lhsT=ones_t[0:1, 0:64], rhs=rowsum[:],
                        start=True, stop=True,
                    )
                    rb = smpool.tile([64, 512], f32, tag="rb")
                    nc.vector.reciprocal_approx_fast(rb[:], rbp[:])
                    nc.vector.tensor_mul(
                        attnT[hp:hp + 64, sq * 512:(sq + 1) * 512],
                        xps[0:64, :], rb[:],
                    )
                if sq % 2 == 1:
                    c = b * 2 + sq // 2
                    aouts[c] = emit_exchange(c, attnT, (sq // 2) * 1024)
                if b == 1:
                    # slot the already-exchanged chunks' output projections
                    # between attention blocks: after sq0 -> wo(0), sq1 ->
                    # wo(1), sq2 -> wo(2); wo(3) runs at the end.
                    if sq < 3:
                        emit_wo(sq, aouts[sq])

        emit_wo(3, aouts[3])

    nc.compile()
    return nc


def _get_nc():
    if "nc" not in _cache:
        _cache["nc"] = _build()
    return _cache["nc"]


def kernel(query, key, value, Wq, bq, Wk, bk, Wv, bv, Wo, bo, trace=False):
    import ml_dtypes
    from concourse.bass_utils import run_bass_kernel_spmd

    bf = ml_dtypes.bfloat16
    nc = _get_nc()

    q = np.ascontiguousarray(
        np.asarray(query, np.float32).reshape(TOK, D).T.astype(bf))
    k = np.ascontiguousarray(
        np.asarray(key, np.float32).reshape(TOK, D).T.astype(bf))
    v = np.ascontiguousarray(
        np.asarray(value, np.float32).reshape(TOK, D).T.astype(bf))
    Wq = np.asarray(Wq, np.float32)
    Wk = np.asarray(Wk, np.float32)
    Wv = np.asarray(Wv, np.float32)
    Wo_b = np.ascontiguousarray(np.asarray(Wo, np.float32).astype(bf))

    in_maps = []
    for r in range(NCORES):
        sl = slice(r * DKC, (r + 1) * DKC)
        in_maps.append({
            "xqT": q, "xkT": k, "xvT": v,
            "wq": np.ascontiguousarray(Wq[:, sl].astype(bf)),
            "wk": np.ascontiguousarray(Wk[:, sl].astype(bf)),
            "wv": np.ascontiguousarray(Wv[:, sl].astype(bf)),
            "wo": Wo_b,
            "bq": np.ascontiguousarray(np.asarray(bq, np.float32)[sl, None]),
            "bk": np.ascontiguousarray(np.asarray(bk, np.float32)[sl, None]),
            "bv": np.ascontiguousarray(
                np.asarray(bv, np.float32)[None, sl].astype(bf)),
        })

    res = run_bass_kernel_spmd(nc, in_maps, list(range(NCORES)), trace=trace)
    _cache["last_results"] = res

    out = np.empty((TOK, D), np.float32)
    for r in range(NCORES):
        o = np.asarray(res.results[r]["out"])
        for c in range(TOK // 1024):
            out[c * 1024 + r * 128: c * 1024 + (r + 1) * 128] = \
                o[c * 128:(c + 1) * 128]
    out = out + np.asarray(bo, np.float32)[None, :]
    return out.reshape(B, S, D)
